# revision 1
# baseline (speedup 1.0000x reference)
"""Trainium2 Bass kernel for nn_Composer (mode-energy softmax).

Math (reference, train path):
    mode_min = min_n phi[n, :]                       (global batch min, per mode)
    phic     = phi - mode_min + e                    (clamp at e is a no-op: min exact)
    L        = ln(phic)
    E[n,i]   = sum_j phic_i^G_ij * phic_j^(1-G_ij)   (G = symmetrized Gamma)
             = sum_j exp( G_ij*L_i + (1-G_ij)*L_j )   for j != i, diag handled below
    out      = (softmax(-t*E), log_softmax(-t*E)),  t = 1/sqrt(32)

Key structure exploited: the exponent X[n,i,j] = G_ij*L_i(n) + (1-G_ij)*L_j(n)
is LINEAR in L(n) -> one PE matmul with a constant stationary A built from G
computes X for 4 i-values x 32 j at once ([128, F] per matmul). Then one ACT
exp, then a second PE matmul with a 0/1(-1/32) stationary reduces over j and
accumulates E - mean_i(E) in PSUM. The diagonal j==i lands on exp(L_i)=phic_i,
exactly the "+ (1+w_i)*phic_i" diag term needs (w folded into the reducer).

Per 512-sample chunk: PE transposes phi into mode-major layout (2x replicated
on partition strips for K=32 row-tiled matmul concurrency), ACT computes
L = ln(phi_T + (e - mode_min)) with a per-partition bias, DVE splits L into
hi/lo fp32r halves (11-bit mantissa each) so the 3-pass f32r X-matmul keeps
~fp32 precision at 1 cycle/row (plain fp32 matmul is 4 cycles/row), ACT does
the big exp (PSUM->SBUF, fp32r out), PE reduces, and a small transposed tail
does the log-softmax with exact max-subtraction.

Two SPMD launches on 8 cores (data-parallel over the batch):
  launch 1: per-core per-mode min -> host combines 8x[32] -> mode_min
  launch 2: the main kernel above with bias = e - mode_min baked per-partition
"""

import sys

sys.path.insert(0, "/opt/trn_rl_repo")

import numpy as np
import concourse.bass as bass
import concourse.tile as tile
from concourse import mybir
from concourse.bass_utils import run_bass_kernel_spmd
from concourse.masks import make_identity

N_CORES = 8
N_BATCH = 131072
M = 32
NS = N_BATCH // N_CORES  # 16384 samples per core
F = 512  # samples per chunk
NCHUNK = NS // F  # 32
KREP = F // 128  # 4 sub-tiles of 128 samples per chunk
E_CONST = float(np.e)
NEG_T = -1.0 / float(np.sqrt(M))

f32 = mybir.dt.float32
f32r = mybir.dt.float32r
AF = mybir.ActivationFunctionType
ALU = mybir.AluOpType


# ----------------------------------------------------------------------------
# helpers
# ----------------------------------------------------------------------------
def _split_sync_waits(nc, max_waits=1):
    """This container's walrus rejects >1 sync wait per instruction in some
    templates; hoist excess waits onto same-engine NOPs placed just before."""
    for fn in nc.m.functions:
        for bb in fn.blocks:
            new_list = []
            for ins in bb.instructions:
                si = getattr(ins, "sync_info", None)
                waits = list(si.on_wait) if si is not None else []
                if len(waits) > max_waits:
                    rest = waits[max_waits:]
                    del si.on_wait[max_waits:]
                    k = 0
                    while rest:
                        chunk, rest = rest[:max_waits], rest[max_waits:]
                        new_list.append(
                            mybir.InstNoOp(
                                name=f"{ins.name}-ws{k}",
                                engine=ins.engine,
                                ins=[],
                                outs=[],
                                sync_info=mybir.SyncInfo(on_wait=chunk, on_update=[]),
                            )
                        )
                        k += 1
                new_list.append(ins)
            bb.instructions[:] = new_list


def _trunc11(x):
    """Round fp32 magnitudes to 11 explicit mantissa bits (f32r precision)."""
    xi = np.ascontiguousarray(x, dtype=np.float32).view(np.uint32).astype(np.int64)
    shift = 23 - 11
    xi = ((xi + (1 << (shift - 1))) >> shift) << shift
    return xi.astype(np.uint32).view(np.float32)


def _rep_free(ap, rep):
    """Insert a stride-0 'repeat' axis right after the partition dim."""
    return bass.AP(tensor=ap.tensor, offset=ap.offset, ap=[ap.ap[0], [0, rep]] + list(ap.ap[1:]))


def _bcast_inner(ap, rep):
    """[P, a] -> [P, a, rep] with stride-0 innermost axis."""
    return bass.AP(tensor=ap.tensor, offset=ap.offset, ap=list(ap.ap) + [[0, rep]])


# ----------------------------------------------------------------------------
# launch 1: per-core per-mode min
# ----------------------------------------------------------------------------
def build_min_nc():
    nc = bass.Bass()
    phi_d = nc.dram_tensor("phi", [NS, M], f32, kind="ExternalInput")
    out_d = nc.dram_tensor("pmin", [M, 1], f32, kind="ExternalOutput")

    NT = 4  # tiles of [128, 128 rows x 32 modes]
    ROWS = NS // NT // 128  # rows per partition per tile = 32

    with tile.TileContext(nc) as tc:
        with (
            tc.tile_pool(name="sb", bufs=2) as sb,
            tc.tile_pool(name="consts", bufs=1) as consts,
            tc.tile_pool(name="ps", bufs=1, space="PSUM") as ps,
        ):
            ident = consts.tile([128, 128], f32)
            make_identity(nc, ident)
            mall = consts.tile([128, M], f32)
            for t in range(NT):
                xt = sb.tile([128, ROWS * M], f32, tag="xt")
                nc.sync.dma_start(
                    xt[:],
                    phi_d[t * (NS // NT) : (t + 1) * (NS // NT), :].rearrange(
                        "(p q) i -> p (q i)", p=128
                    ),
                )
                mt = sb.tile([128, M], f32, tag="mt")
                # view free dim as (q, i) -> reduce over q (strided inner axis)
                nc.vector.tensor_reduce(
                    mt[:],
                    xt[:].rearrange("p (q i) -> p i q", i=M),
                    axis=mybir.AxisListType.X,
                    op=ALU.min,
                )
                if t == 0:
                    nc.vector.tensor_copy(mall[:], mt[:])
                else:
                    nc.vector.tensor_tensor(mall[:], mall[:], mt[:], op=ALU.min)
            pt = ps.tile([M, 128], f32)
            nc.tensor.transpose(pt[:], mall[:], ident[:])
            pm = consts.tile([M, 1], f32)
            nc.vector.tensor_reduce(pm[:], pt[:], axis=mybir.AxisListType.X, op=ALU.min)
            nc.sync.dma_start(out_d[:], pm[:])
    _split_sync_waits(nc)
    return nc


# ----------------------------------------------------------------------------
# launch 2: main kernel
# ----------------------------------------------------------------------------
NSTRIP = 2  # K=32 row-strips used concurrently by the X matmuls
X_PASSES = 3  # 3 = hi/lo split (full precision); 1 = single pass (ablation)
TAIL_STUB = False  # True: skip softmax tail (timing ablation only)
EXP_HALF = False  # True: exp only half the X tile (timing ablation only)


def build_main_nc(repeat=1):
    nc = bass.Bass()
    phi_d = nc.dram_tensor("phi", [NS, M], f32, kind="ExternalInput")
    bias_d = nc.dram_tensor("bias128", [128, 1], f32, kind="ExternalInput")
    xhi_d = nc.dram_tensor("xhi", [128, M // 4 // NSTRIP, 128], f32, kind="ExternalInput")
    xlo_d = nc.dram_tensor("xlo", [128, M // 4 // NSTRIP, 128], f32, kind="ExternalInput")
    red_d = nc.dram_tensor("red", [128, M // 4, M], f32, kind="ExternalInput")
    alphas_d = nc.dram_tensor("alphas", [NS, M], f32, kind="ExternalOutput")
    logits_d = nc.dram_tensor("logits", [NS, M], f32, kind="ExternalOutput")

    NPAIR = M // 4 // NSTRIP  # 4 groups of NSTRIP iqs

    with tile.TileContext(nc) as tc:
        with (
            tc.tile_pool(name="consts", bufs=1) as consts,
            tc.tile_pool(name="inb", bufs=3) as inb,
            tc.tile_pool(name="lt", bufs=2) as ltp,
            tc.tile_pool(name="tt", bufs=3) as ttp,
            tc.tile_pool(name="tail", bufs=2) as tailp,
            tc.tile_pool(name="outb", bufs=3) as outb,
            tc.tile_pool(name="pt4", bufs=1, space="PSUM") as pt4p,
            tc.tile_pool(name="xps", bufs=2, space="PSUM") as xpsp,
            tc.tile_pool(name="eps", bufs=2, space="PSUM") as epsp,
            tc.tile_pool(name="ztps", bufs=1, space="PSUM") as ztpsp,
        ):
            ident = consts.tile([128, 128], f32)
            make_identity(nc, ident)
            bias_s = consts.tile([128, 1], f32)
            nc.sync.dma_start(bias_s[:], bias_d[:])
            xhi_s = consts.tile([128, NPAIR, 128], f32)
            xlo_s = consts.tile([128, NPAIR, 128], f32)
            red_s = consts.tile([128, M // 4, M], f32)
            nc.sync.dma_start(xhi_s[:], xhi_d[:])
            nc.sync.dma_start(xlo_s[:], xlo_d[:])
            nc.sync.dma_start(red_s[:], red_d[:])
            # round constants into f32r tiles (device rounding = the producer
            # the BIR verifier requires for f32r matmul operands)
            xhi_r = consts.tile([128, NPAIR, 128], f32r)
            xlo_r = consts.tile([128, NPAIR, 128], f32r)
            red_r = consts.tile([128, M // 4, M], f32r)
            nc.vector.tensor_copy(xhi_r[:], xhi_s[:])
            nc.vector.tensor_copy(xlo_r[:], xlo_s[:])
            nc.vector.tensor_copy(red_r[:], red_s[:])

            for _rep in range(repeat):
              for c in range(NCHUNK):
                r0 = c * F
                # ---- load [F, 32] as [128, (KREP, 32)]
                phia = inb.tile([128, KREP, M], f32, tag="phia")
                nc.sync.dma_start(
                    phia[:],
                    phi_d[r0 : r0 + F, :].rearrange("(k p) i -> p k i", p=128),
                )
                pt4 = pt4p.tile([32, F], f32, tag="pt4")
                for k in range(KREP):
                    nc.tensor.transpose(
                        pt4[0:32, k * 128 : (k + 1) * 128],
                        phia[:, k, :],
                        ident[:],
                    )
                # ---- L = ln(phi_T + bias) on strip 0, replicate to other
                # strips via SBUF->SBUF DMA, then hi/lo f32r split
                lt4 = ltp.tile([NSTRIP * 32, F], f32, tag="lt4")
                nc.scalar.activation(
                    lt4[0:32, :], pt4[:], AF.Ln, bias=bias_s[:32], scale=1.0
                )
                for s in range(1, NSTRIP):
                    nc.sync.dma_start(lt4[32 * s : 32 * s + 32, :], lt4[0:32, :])
                lhi = ltp.tile([NSTRIP * 32, F], f32r, tag="lhi")
                llo = ltp.tile([NSTRIP * 32, F], f32r, tag="llo")
                nc.vector.tensor_copy(lhi[:], lt4[:])
                nc.vector.tensor_tensor(llo[:], lt4[:], lhi[:].bitcast(f32), op=ALU.subtract)

                for t in range(NPAIR):
                    xps = xpsp.tile([128, NSTRIP * F], f32, tag="xps")
                    for p_, (sa, sb_) in enumerate(
                        [(xhi_r, lhi), (xhi_r, llo), (xlo_r, lhi)][:X_PASSES]
                    ):
                        for s in range(NSTRIP):
                            nc.tensor.matmul(
                                xps[:, s * F : (s + 1) * F],
                                sa[s * 32 : (s + 1) * 32, t, :],
                                sb_[s * 32 : (s + 1) * 32, :],
                                start=(p_ == 0),
                                stop=(p_ == X_PASSES - 1),
                                tile_position=(32 * s, 0),
                                skip_group_check=True,
                            )
                    tt = ttp.tile([128, NSTRIP * F], f32r, tag="tt")
                    if EXP_HALF:
                        nc.scalar.activation(
                            tt[:, : NSTRIP * F // 2], xps[:, : NSTRIP * F // 2], AF.Exp
                        )
                    else:
                        nc.scalar.activation(tt[:], xps[:], AF.Exp)
                    if t == 0:
                        eps = epsp.tile([M, F], f32, tag="eps")
                    for s in range(NSTRIP):
                        iq = NSTRIP * t + s
                        nc.tensor.matmul(
                            eps[:],
                            red_r[:, iq, :],
                            tt[:, s * F : (s + 1) * F],
                            start=(t == 0 and s == 0),
                            stop=(t == NPAIR - 1 and s == NSTRIP - 1),
                            skip_group_check=True,
                        )

                # ---- tail: z = -t*(E - mean); transpose to sample-major;
                # log-softmax with exact max subtraction
                zs = tailp.tile([M, F], f32, tag="zs")
                nc.scalar.mul(zs[:], eps[:], NEG_T)
                ztps = ztpsp.tile([128, KREP * M], f32, tag="ztps")
                for k in range(KREP):
                    nc.tensor.transpose(
                        ztps[:, k * M : (k + 1) * M],
                        zs[:, k * 128 : (k + 1) * 128],
                        ident[:M, :M],
                    )
                zt = tailp.tile([128, KREP * M], f32, tag="zt")
                nc.scalar.copy(zt[:], ztps[:])
                if TAIL_STUB:
                    nc.sync.dma_start(
                        logits_d[r0 : r0 + F, :].rearrange("(k p) i -> p k i", p=128),
                        zt[:],
                    )
                    nc.sync.dma_start(
                        alphas_d[r0 : r0 + F, :].rearrange("(k p) i -> p k i", p=128),
                        zt[:],
                    )
                    continue
                zt_v = zt[:].rearrange("p (k i) -> p k i", i=M)
                m4 = tailp.tile([128, KREP], f32, tag="m4")
                nc.vector.tensor_reduce(m4[:], zt_v, axis=mybir.AxisListType.X, op=ALU.max)
                x2 = tailp.tile([128, KREP, M], f32, tag="x2")
                nc.vector.tensor_tensor(
                    x2[:], zt_v, _bcast_inner(m4[:], M), op=ALU.subtract
                )
                p4 = tailp.tile([128, KREP, M], f32, tag="p4")
                nc.scalar.activation(p4[:], x2[:], AF.Exp)
                s2 = tailp.tile([128, KREP], f32, tag="s2")
                nc.vector.tensor_reduce(s2[:], p4[:], axis=mybir.AxisListType.X, op=ALU.add)
                ls = tailp.tile([128, KREP], f32, tag="ls")
                nc.scalar.activation(ls[:], s2[:], AF.Ln)
                logit_s = outb.tile([128, KREP, M], f32, tag="logit_s")
                nc.vector.tensor_tensor(
                    logit_s[:], x2[:], _bcast_inner(ls[:], M), op=ALU.subtract
                )
                alpha_s = outb.tile([128, KREP, M], f32, tag="alpha_s")
                nc.scalar.activation(alpha_s[:], logit_s[:], AF.Exp)
                nc.sync.dma_start(
                    logits_d[r0 : r0 + F, :].rearrange("(k p) i -> p k i", p=128),
                    logit_s[:],
                )
                nc.sync.dma_start(
                    alphas_d[r0 : r0 + F, :].rearrange("(k p) i -> p k i", p=128),
                    alpha_s[:],
                )
    _split_sync_waits(nc)
    return nc


# ----------------------------------------------------------------------------
# host-side constants from Gamma / w
# ----------------------------------------------------------------------------
def build_stationaries(Gamma, w):
    idx = np.arange(M)
    G = np.where(idx[:, None] < idx[None, :], Gamma, Gamma.T).astype(np.float64)

    # X stationaries: A_iq[k, 32g + j] = G[i,j]*[k==i] + (1-G[i,j])*[k==j],
    # i = 4*iq + g, packed NSTRIP iqs per 128-partition tile (strip s = iq 2t+s)
    A = np.zeros((M // 4, M, 128), dtype=np.float64)
    for iq in range(M // 4):
        for g in range(4):
            i = 4 * iq + g
            for j in range(M):
                col = 32 * g + j
                A[iq, i, col] += G[i, j]
                A[iq, j, col] += 1.0 - G[i, j]
    A = A.astype(np.float32)
    A_hi = _trunc11(A)
    A_lo = (A - A_hi).astype(np.float32)

    NPAIR = M // 4 // NSTRIP
    xhi = np.zeros((128, NPAIR, 128), dtype=np.float32)
    xlo = np.zeros((128, NPAIR, 128), dtype=np.float32)
    for t in range(NPAIR):
        for s in range(NSTRIP):
            iq = NSTRIP * t + s
            xhi[32 * s : 32 * s + 32, t, :] = A_hi[iq]
            xlo[32 * s : 32 * s + 32, t, :] = A_lo[iq]

    # reducer: E'_m = sum_iq sum_k S_iq[k, m] T_iq[k, n] = E_m - mean_i(E_i)
    # S_iq[k, m] = [m == 4iq + k//32] - 1/32  (+ w_m at the diagonal entry)
    red = np.zeros((128, M // 4, M), dtype=np.float32)
    for iq in range(M // 4):
        S = np.full((128, M), -1.0 / M, dtype=np.float64)
        for g in range(4):
            i = 4 * iq + g
            S[32 * g : 32 * g + 32, i] += 1.0
            S[32 * g + i, i] += float(w[i])
        red[:, iq, :] = S.astype(np.float32)
    return xhi, xlo, red


_NC_CACHE = {}


def _get_ncs():
    if "min" not in _NC_CACHE:
        _NC_CACHE["min"] = build_min_nc()
        _NC_CACHE["main"] = build_main_nc()
    return _NC_CACHE["min"], _NC_CACHE["main"]


def kernel(phi, Gamma, w):
    phi = np.ascontiguousarray(np.asarray(phi), dtype=np.float32)
    Gamma = np.asarray(Gamma, dtype=np.float32)
    w = np.asarray(w, dtype=np.float32)
    assert phi.shape == (N_BATCH, M)

    nc_min, nc_main = _get_ncs()
    core_ids = list(range(N_CORES))
    shards = [phi[c * NS : (c + 1) * NS] for c in range(N_CORES)]

    # launch 1: global per-mode min
    res1 = run_bass_kernel_spmd(
        nc_min, [{"phi": s} for s in shards], core_ids=core_ids
    ).results
    mode_min = np.min(
        np.stack([r["pmin"][:, 0] for r in res1], axis=0), axis=0
    ).astype(np.float32)

    # launch 2: main kernel
    bias128 = np.tile((E_CONST - mode_min).reshape(1, M), (4, 1)).reshape(128, 1)
    xhi, xlo, red = build_stationaries(Gamma, w)
    in_maps = [
        {"phi": s, "bias128": bias128, "xhi": xhi, "xlo": xlo, "red": red}
        for s in shards
    ]
    res2 = run_bass_kernel_spmd(nc_main, in_maps, core_ids=core_ids).results
    alphas = np.concatenate([r["alphas"] for r in res2], axis=0)
    logits = np.concatenate([r["logits"] for r in res2], axis=0)
    return alphas, logits

